# revision 1
# baseline (speedup 1.0000x reference)
"""DLRM pairwise-interaction kernel for Trainium2 (8 NeuronCores).

Computes, for each batch b: Z_b = X_b @ X_b^T (X_b is [64, 256]) and emits the
strict lower triangle row-major -> [B, 2016] fp32.

Strategy (pure data parallel over B, 1024 batches per core):
  - fp32 inputs loaded with large HWDGE DMAs (scalar ring), cast to fp16 on
    DVE/ACT (alternating) (fp16 keeps the PE at 1 cycle/row; dot-product accumulation stays
    fp32 in PSUM so the end-to-end relative error is ~3e-4).
  - PE-transposes pairs of batches ([128,128] fp16 tiles, identity matmul) to
    put the contraction dim (d) on partitions; DVE evacuates PSUM->SBUF.
  - Per batch: Z = XT^T @ XT as two K=128 accumulating matmuls; two batches
    share the 128 weight columns (column-tiled via output base partition 0/64),
    eight pairs of Z accumulate into one [128, 512] fp32 PSUM bank before a
    single DVE copy to SBUF.
  - Triangle compaction: per triangle row i, two SBUF->SBUF DMAs (one per
    128-batch half) on the scalar HWDGE ring move row i for all 256 batches of
    a supertile into a dense [128, 2*2016] image; two 1 MB contiguous DMAs
    store it to HBM.
"""
import sys
import numpy as np

sys.path.insert(0, "/opt/trn_rl_repo")

import concourse.bass as bass
import concourse.mybir as mybir
import concourse.tile as tile
from concourse.vector_clock import ScopedClock

F32 = mybir.dt.float32
F16 = mybir.dt.float16

B, N, D, TRI = 8192, 64, 256, 2016
NCORES = 8
NB = B // NCORES           # batches per core
ST_PAIRS = 128             # pairs per supertile (=256 batches)
CHUNK_PAIRS = 16           # pairs per input DMA (2 MB fp32 read)

# ---------------------------------------------------------------------------
# Workaround for walrus builds that only accept ONE sync-wait per instruction:
# hoist all-but-one wait onto NoOp instructions committed just before, on the
# same engine (same-engine program order preserves semantics).
# ---------------------------------------------------------------------------
_orig_commit = tile.TileContext._commit_instruction


def _split_waits(self, inst):
    si = getattr(inst, "sync_info", None)
    if si is None or not si.on_wait or len(si.on_wait) <= 1:
        return
    if inst.engine == mybir.EngineType.Unassigned:
        return
    waits = list(si.on_wait)
    inst.sync_info = mybir.SyncInfo(on_wait=[waits[-1]], on_update=list(si.on_update))
    for w in waits[:-1]:
        nop = mybir.InstNoOp(name=f"{inst.name}-wsplit-{w.id}", ins=[], outs=[])
        nop.engine = inst.engine
        nop.sync_info = mybir.SyncInfo(on_wait=[w], on_update=[])
        _orig_commit(self, nop, lazy_reg_writes=False)


def _commit_instruction_split(self, inst, lazy_reg_writes=True):
    _split_waits(self, inst)
    return _orig_commit(self, inst, lazy_reg_writes=lazy_reg_writes)


def _drain_and_barrier_split(self, tick_clock, wait_clock):
    drain_inst = self.nc.sync.drain()
    wait_clock.add_sem_waits(
        drain_inst.ins, ScopedClock({None: tick_clock.global_clock})
    )
    si = drain_inst.ins.sync_info
    if si is not None and si.on_wait and len(si.on_wait) > 1:
        waits = list(si.on_wait)
        drain_inst.ins.sync_info = mybir.SyncInfo(
            on_wait=[waits[0]], on_update=list(si.on_update)
        )
        for w in waits[1:]:
            nop = self.nc.sync.nop(nofuse=True)
            nop.ins.sync_info = mybir.SyncInfo(on_wait=[w], on_update=[])

    self.nc.all_engine_barrier()
    assert self.sems is not None
    popped = self.nc._tile_sem_poison_stack.pop()
    assert popped is self._sem_poison
    self.nc.clear_and_free_semaphores(list(self.sems.allocated().values()))
    self.nc.all_engine_barrier()


def _install_tile_workarounds():
    tile.TileContext._commit_instruction = _commit_instruction_split
    tile.TileContext._drain_and_barrier = _drain_and_barrier_split


def build_program(nb=NB, st_pairs=ST_PAIRS, chunk_pairs=CHUNK_PAIRS):
    _install_tile_workarounds()
    npairs = nb // 2
    nst = npairs // st_pairs
    nchunks = st_pairs // chunk_pairs
    assert st_pairs == 128

    nc = bass.Bass("TRN2", target_bir_lowering=False, debug=False,
                   num_devices=NCORES)
    x = nc.dram_tensor("x", [nb, N, D], F32, kind="ExternalInput").ap()
    ident = nc.dram_tensor("ident", [128, 128], F16, kind="ExternalInput").ap()
    y = nc.dram_tensor("y", [nb, TRI], F32, kind="ExternalOutput").ap()
    xflat = x.rearrange("b n d -> (b n) d")

    with tile.TileContext(nc) as tc:
        with (
            tc.tile_pool(name="const", bufs=1) as constp,
            tc.tile_pool(name="xf32", bufs=2) as xf32p,
            tc.tile_pool(name="xin", bufs=3) as xinp,
            tc.tile_pool(name="xt", bufs=3) as xtp_sb,
            tc.tile_pool(name="zsb", bufs=2) as zsbp,
            tc.tile_pool(name="osb", bufs=2) as osbp,
            tc.tile_pool(name="xtps", bufs=3, space="PSUM") as xtps,
            tc.tile_pool(name="zps", bufs=3, space="PSUM") as zps,
        ):
            ident_sb = constp.tile([128, 128], F16)
            nc.sync.dma_start(ident_sb[:], ident[:])

            for s in range(nst):
                # ---- load X (fp32, sync HWDGE ring) + cast on DVE/ACT ----
                xbufs = []
                for c in range(nchunks):
                    xf = xf32p.tile([128, chunk_pairs * 256], F32, tag="xf32")
                    row0 = (s * st_pairs + c * chunk_pairs) * 128
                    src = xflat[row0:row0 + chunk_pairs * 128, :].rearrange(
                        "(l p) d -> p l d", p=128)
                    nc.sync.dma_start(
                        xf[:].rearrange("p (l d) -> p l d", d=256), src)
                    xb = xinp.tile([128, chunk_pairs * 256], F16, tag="xin")
                    if c % 2 == 0:
                        nc.vector.tensor_copy(xb[:], xf[:])
                    else:
                        nc.scalar.copy(xb[:], xf[:])
                    xbufs.append(xb)

                z_sb = zsbp.tile([128, st_pairs * 64], F32, tag="zsb")
                for q8 in range(st_pairs // 8):
                    zp = zps.tile([128, 512], F32, tag="zps")
                    for half in range(2):
                        q4 = q8 * 2 + half
                        xtp = xtps.tile([128, 1024], F16, tag="xtps")
                        for pl in range(4):
                            l = q4 * 4 + pl
                            cidx, lc = divmod(l, chunk_pairs)
                            for c in range(2):
                                nc.tensor.transpose(
                                    xtp[:, pl * 256 + c * 128:pl * 256 + (c + 1) * 128],
                                    xbufs[cidx][:, lc * 256 + c * 128:lc * 256 + (c + 1) * 128],
                                    ident_sb[:])
                        xt = xtp_sb.tile([128, 1024], F16, tag="xt")
                        nc.vector.tensor_copy(xt[:], xtp[:])
                        for pl in range(4):
                            slot = half * 4 + pl
                            q0 = pl * 256
                            q1 = pl * 256 + 128
                            zsl = zp[:, slot * 64:(slot + 1) * 64]
                            nc.tensor.matmul(zsl[0:64, :], xt[:, q0:q0 + 64],
                                             xt[:, q0:q0 + 64],
                                             start=True, stop=False,
                                             skip_group_check=True)
                            nc.tensor.matmul(zsl[64:128, :], xt[:, q0 + 64:q0 + 128],
                                             xt[:, q0 + 64:q0 + 128],
                                             start=True, stop=False,
                                             skip_group_check=True)
                            nc.tensor.matmul(zsl[0:64, :], xt[:, q1:q1 + 64],
                                             xt[:, q1:q1 + 64],
                                             start=False, stop=True,
                                             skip_group_check=True)
                            nc.tensor.matmul(zsl[64:128, :], xt[:, q1 + 64:q1 + 128],
                                             xt[:, q1 + 64:q1 + 128],
                                             start=False, stop=True,
                                             skip_group_check=True)
                    nc.vector.tensor_copy(z_sb[:, q8 * 512:(q8 + 1) * 512], zp[:])

                # ---- compaction: two DMAs (halves) per triangle row i ------
                out_sb = osbp.tile([128, 2 * TRI], F32, tag="osb")
                zr = z_sb[:].rearrange("(g q) (h l2 j) -> g q h l2 j",
                                       g=2, h=2, j=64)
                orr = out_sb[:].rearrange("p (h t) -> p h t", h=2)
                for i in range(1, 64):
                    off = i * (i - 1) // 2
                    eng = nc.scalar if i % 2 == 0 else nc.sync
                    for h in range(2):
                        eng.dma_start(
                            orr[:, h, off:off + i],
                            zr[:, i, h, :, 0:i])

                # ---- dense store: 2 x 1 MB ---------------------------------
                base = s * 2 * st_pairs
                for h in range(2):
                    ydst = y[base + h * 128: base + (h + 1) * 128, :].rearrange(
                        "(l2 g) t -> g l2 t", g=2)
                    nc.sync.dma_start(ydst, orr[:, h, :])
    return nc


_PROGRAM_CACHE = {}


def _get_program():
    if "nc" not in _PROGRAM_CACHE:
        _PROGRAM_CACHE["nc"] = build_program()
    return _PROGRAM_CACHE["nc"]


def kernel(inputs):
    from concourse.bass_utils import run_bass_kernel_spmd

    x = np.asarray(inputs, dtype=np.float32)
    assert x.shape == (B, N, D), x.shape
    nc = _get_program()
    eye = np.eye(128, dtype=np.float16)
    in_maps = [
        {"x": np.ascontiguousarray(x[i * NB:(i + 1) * NB]), "ident": eye}
        for i in range(NCORES)
    ]
    res = run_bass_kernel_spmd(nc, in_maps, list(range(NCORES)))
    out = np.concatenate([res.results[i]["y"] for i in range(NCORES)], axis=0)
    return out.astype(np.float32, copy=False)



# revision 6
# speedup vs baseline: 1.8540x; 1.8540x over previous
"""DLRM pairwise-interaction kernel for Trainium2 (8 NeuronCores).

Computes, for each batch b: Z_b = X_b @ X_b^T (X_b is [64, 256]) and emits the
strict lower triangle row-major -> [B, 2016] fp32.

Strategy (pure data parallel over B, 1024 batches per core):
  - fp32 inputs loaded with large HWDGE DMAs (scalar ring), cast to fp16 on
    DVE/ACT (alternating) (fp16 keeps the PE at 1 cycle/row; dot-product accumulation stays
    fp32 in PSUM so the end-to-end relative error is ~3e-4).
  - PE-transposes pairs of batches ([128,128] fp16 tiles, identity matmul) to
    put the contraction dim (d) on partitions; DVE evacuates PSUM->SBUF.
  - Per batch: Z = XT^T @ XT as two K=128 accumulating matmuls; two batches
    share the 128 weight columns (column-tiled via output base partition 0/64),
    eight pairs of Z accumulate into one [128, 512] fp32 PSUM bank before a
    single DVE copy to SBUF.
  - Stage C: 64 more PE transposes per supertile flip Z from [part=(g,row),
    free=(pair,col)] to [part=pair, free=(g,row,col)] (f16), putting batches on
    partitions with rows contiguous in the free dim.
  - Triangle compaction: 126 per-row engine copies (DVE/ACT/GpSimd round-robin,
    f16->f32 cast) build the packed [128, 2*2016] image entirely on compute
    engines -- no small-descriptor DMAs.
  - Store: one 2 MB SWDGE (gpsimd) DMA per supertile; its 128 x 16 KB
    descriptors spread across all 16 SDMA engines, unlike HWDGE dynamic DMAs
    which serialize on 2 engines.
"""
import sys
import numpy as np

sys.path.insert(0, "/opt/trn_rl_repo")

import concourse.bass as bass
import concourse.mybir as mybir
import concourse.tile as tile
from concourse.vector_clock import ScopedClock

F32 = mybir.dt.float32
F16 = mybir.dt.float16

B, N, D, TRI = 8192, 64, 256, 2016
NCORES = 8
NB = B // NCORES           # batches per core
ST_PAIRS = 128             # pairs per supertile (=256 batches)
CHUNK_PAIRS = 16           # pairs per input DMA (2 MB fp32 read)

# ---------------------------------------------------------------------------
# Workaround for walrus builds that only accept ONE sync-wait per instruction:
# hoist all-but-one wait onto NoOp instructions committed just before, on the
# same engine (same-engine program order preserves semantics).
# ---------------------------------------------------------------------------
_orig_commit = tile.TileContext._commit_instruction


def _split_waits(self, inst):
    si = getattr(inst, "sync_info", None)
    if si is None or not si.on_wait or len(si.on_wait) <= 1:
        return
    if inst.engine == mybir.EngineType.Unassigned:
        return
    waits = list(si.on_wait)
    inst.sync_info = mybir.SyncInfo(on_wait=[waits[-1]], on_update=list(si.on_update))
    for w in waits[:-1]:
        nop = mybir.InstNoOp(name=f"{inst.name}-wsplit-{w.id}", ins=[], outs=[])
        nop.engine = inst.engine
        nop.sync_info = mybir.SyncInfo(on_wait=[w], on_update=[])
        _orig_commit(self, nop, lazy_reg_writes=False)


def _commit_instruction_split(self, inst, lazy_reg_writes=True):
    _split_waits(self, inst)
    return _orig_commit(self, inst, lazy_reg_writes=lazy_reg_writes)


def _drain_and_barrier_split(self, tick_clock, wait_clock):
    drain_inst = self.nc.sync.drain()
    wait_clock.add_sem_waits(
        drain_inst.ins, ScopedClock({None: tick_clock.global_clock})
    )
    si = drain_inst.ins.sync_info
    if si is not None and si.on_wait and len(si.on_wait) > 1:
        waits = list(si.on_wait)
        drain_inst.ins.sync_info = mybir.SyncInfo(
            on_wait=[waits[0]], on_update=list(si.on_update)
        )
        for w in waits[1:]:
            nop = self.nc.sync.nop(nofuse=True)
            nop.ins.sync_info = mybir.SyncInfo(on_wait=[w], on_update=[])

    self.nc.all_engine_barrier()
    assert self.sems is not None
    popped = self.nc._tile_sem_poison_stack.pop()
    assert popped is self._sem_poison
    self.nc.clear_and_free_semaphores(list(self.sems.allocated().values()))
    self.nc.all_engine_barrier()


def _install_tile_workarounds():
    tile.TileContext._commit_instruction = _commit_instruction_split
    tile.TileContext._drain_and_barrier = _drain_and_barrier_split


def build_program(nb=NB, st_pairs=ST_PAIRS, chunk_pairs=CHUNK_PAIRS):
    _install_tile_workarounds()
    npairs = nb // 2
    nst = npairs // st_pairs
    nchunks = st_pairs // chunk_pairs
    assert st_pairs == 128

    nc = bass.Bass("TRN2", target_bir_lowering=False, debug=False,
                   num_devices=NCORES)
    x = nc.dram_tensor("x", [nb, N, D], F32, kind="ExternalInput").ap()
    ident = nc.dram_tensor("ident", [128, 128], F16, kind="ExternalInput").ap()
    y = nc.dram_tensor("y", [nb, TRI], F32, kind="ExternalOutput").ap()
    xflat = x.rearrange("b n d -> (b n) d")

    with tile.TileContext(nc) as tc:
        with (
            tc.tile_pool(name="const", bufs=1) as constp,
            tc.tile_pool(name="xf32", bufs=2) as xf32p,
            tc.tile_pool(name="xin", bufs=3) as xinp,
            tc.tile_pool(name="xt", bufs=3) as xtp_sb,
            tc.tile_pool(name="zsb", bufs=2) as zsbp,
            tc.tile_pool(name="tgi", bufs=2) as tgip,
            tc.tile_pool(name="osb", bufs=2) as osbp,
            tc.tile_pool(name="xtps", bufs=3, space="PSUM") as xtps,
            tc.tile_pool(name="zps", bufs=3, space="PSUM") as zps,
            tc.tile_pool(name="ctps", bufs=2, space="PSUM") as ctps,
        ):
            ident_sb = constp.tile([128, 128], F16)
            nc.sync.dma_start(ident_sb[:], ident[:])

            for s in range(nst):
                # ---- load X (fp32, sync HWDGE ring) + cast on DVE/ACT ----
                xbufs = []
                for c in range(nchunks):
                    xf = xf32p.tile([128, chunk_pairs * 256], F32, tag="xf32")
                    row0 = (s * st_pairs + c * chunk_pairs) * 128
                    src = xflat[row0:row0 + chunk_pairs * 128, :].rearrange(
                        "(l p) d -> p l d", p=128)
                    nc.sync.dma_start(
                        xf[:].rearrange("p (l d) -> p l d", d=256), src)
                    xb = xinp.tile([128, chunk_pairs * 256], F16, tag="xin")
                    if c % 2 == 0:
                        nc.vector.tensor_copy(xb[:], xf[:])
                    else:
                        nc.scalar.copy(xb[:], xf[:])
                    xbufs.append(xb)

                z_sb = zsbp.tile([128, st_pairs * 64], F16, tag="zsb")
                for q8 in range(st_pairs // 8):
                    zp = zps.tile([128, 512], F32, tag="zps")
                    for half in range(2):
                        q4 = q8 * 2 + half
                        xtp = xtps.tile([128, 1024], F16, tag="xtps")
                        for pl in range(4):
                            l = q4 * 4 + pl
                            cidx, lc = divmod(l, chunk_pairs)
                            for c in range(2):
                                nc.tensor.transpose(
                                    xtp[:, pl * 256 + c * 128:pl * 256 + (c + 1) * 128],
                                    xbufs[cidx][:, lc * 256 + c * 128:lc * 256 + (c + 1) * 128],
                                    ident_sb[:])
                        xt = xtp_sb.tile([128, 1024], F16, tag="xt")
                        nc.vector.tensor_copy(xt[:], xtp[:])
                        for pl in range(4):
                            slot = half * 4 + pl
                            q0 = pl * 256
                            q1 = pl * 256 + 128
                            zsl = zp[:, slot * 64:(slot + 1) * 64]
                            nc.tensor.matmul(zsl[0:64, :], xt[:, q0:q0 + 64],
                                             xt[:, q0:q0 + 64],
                                             start=True, stop=False,
                                             skip_group_check=True)
                            nc.tensor.matmul(zsl[64:128, :], xt[:, q0 + 64:q0 + 128],
                                             xt[:, q0 + 64:q0 + 128],
                                             start=True, stop=False,
                                             skip_group_check=True)
                            nc.tensor.matmul(zsl[0:64, :], xt[:, q1:q1 + 64],
                                             xt[:, q1:q1 + 64],
                                             start=False, stop=True,
                                             skip_group_check=True)
                            nc.tensor.matmul(zsl[64:128, :], xt[:, q1 + 64:q1 + 128],
                                             xt[:, q1 + 64:q1 + 128],
                                             start=False, stop=True,
                                             skip_group_check=True)
                    nc.vector.tensor_copy(z_sb[:, q8 * 512:(q8 + 1) * 512], zp[:])

                # ---- stage C: PE-transpose Z -> batch on partitions --------
                # z_sb: [part (g,i), free (l, j)] -> T_gi: [part l, free (g,i,j)]
                zr2 = z_sb[:].rearrange("p (l j) -> p j l", j=64)
                t_gi = tgip.tile([128, 2 * 64 * 64], F16, tag="tgi")
                tdst = t_gi[:].rearrange("p (g i j) -> p j g i", g=2, j=64)
                for j4 in range(16):
                    ct = ctps.tile([128, 512], F16, tag="ctps")
                    for jj in range(4):
                        j = j4 * 4 + jj
                        nc.tensor.transpose(
                            ct[:, jj * 128:(jj + 1) * 128], zr2[:, j, :],
                            ident_sb[:])
                    # evac with (jj,g,i) -> (g,i,j) shuffle + f32->f16 cast
                    nc.vector.tensor_copy(
                        tdst[:, j4 * 4:(j4 + 1) * 4, :, :],
                        ct[:].rearrange("p (j g i) -> p j g i", g=2, i=64))

                # ---- compaction: per-row engine copies (f16 -> f32) --------
                out_sb = osbp.tile([128, 2 * TRI], F32, tag="osb")
                engs = (nc.vector, nc.scalar, nc.gpsimd)
                k = 0
                for i in range(1, 64):
                    off = i * (i - 1) // 2
                    for g in range(2):
                        eng = engs[k % 3]
                        k += 1
                        src = t_gi[:, (g * 64 + i) * 64:(g * 64 + i) * 64 + i]
                        dst = out_sb[:, g * TRI + off:g * TRI + off + i]
                        if eng is nc.scalar:
                            eng.copy(dst, src)
                        else:
                            eng.tensor_copy(dst, src)

                # ---- store: one 2 MB SWDGE DMA (spreads over 16 engines) ---
                base = s * 2 * st_pairs
                ydst = y[base: base + 2 * st_pairs, :].rearrange(
                    "(l g) t -> l (g t)", g=2)
                nc.gpsimd.dma_start(ydst, out_sb[:])
    return nc


_PROGRAM_CACHE = {}


def _get_program():
    if "nc" not in _PROGRAM_CACHE:
        _PROGRAM_CACHE["nc"] = build_program()
    return _PROGRAM_CACHE["nc"]


def kernel(inputs):
    from concourse.bass_utils import run_bass_kernel_spmd

    x = np.asarray(inputs, dtype=np.float32)
    assert x.shape == (B, N, D), x.shape
    nc = _get_program()
    eye = np.eye(128, dtype=np.float16)
    in_maps = [
        {"x": np.ascontiguousarray(x[i * NB:(i + 1) * NB]), "ident": eye}
        for i in range(NCORES)
    ]
    res = run_bass_kernel_spmd(nc, in_maps, list(range(NCORES)))
    out = np.concatenate([res.results[i]["y"] for i in range(NCORES)], axis=0)
    return out.astype(np.float32, copy=False)

